# revision 1
# baseline (speedup 1.0000x reference)
"""DiffGLCM Trainium2 kernel: diagonal layout + paired-stationary matmuls.

Reference: per pixel t_j = A_j - A_{j+1} with A = [1, sigma-edges, 0] (the
cumulative soft-binning vector, A_k = sigmoid(640(x - k/64)) with exact
0/1 end rows via shift values -10/+11); GLCM = sum over (1,1)-offset pixel
pairs of outer(t_c, t_p), normalized per image. The kernel computes
S = sum_pairs outer(A_c, A_p) on the PE; GLCM = 2D second difference of S,
applied on host in fp64 (so only one sigmoid pass per pixel is needed).

Layout: the (1,1)-offset pairing maps (r,c) -> (r+1,c+1), which stays on
the same true diagonal D = r-c. Host-side each image is repacked into
x_diag[p, s]: partition p holds diagonals D === p (mod 128), each segment
in increasing c, preceded by a sentinel slot (x = -100, whose entire
sigmoid A-vector is exactly 0). The pair becomes (p,s)~(p,s+1), so the
periph matmul operand is the SAME SBUF tensor at a free-dim +1 offset:
no partition-shifted operand (PE base partition must be 0/32/64) and no
SBUF->SBUF shift copies (~46us/MB, fatal), and no duplicated sigmoid work.

Compute per image (2 per core x 8 cores): A = sigmoid(640(x-shift)) over
[128, 65, 516] bf16, built in 6 chunks (fp32 broadcast-sub split
DVE/GPSIMD, sigmoid on ACT ~0.83ns/elem - the bottleneck engine).
Matmuls are PAIRED on a stationary slot: for odd s', one matmul with
stationary A(s') streams the contiguous moving window [A(s'-1) | A(s') |
A(s'+1)] (N=195): output column-block h=0 accumulates S^T (pair s'-1),
h=2 accumulates S (pair s'), h=1 is a harmless self-product. This loads
each stationary only once and cuts matmul instruction count 4x (1030 ->
516 per core), which measured 109us -> 79us. 4 PSUM accumulator groups
(by s' half/quarter) keep fp32 accumulation error small; host sums accs
in fp64, combines S^T + S halves, applies the second difference and
per-image normalization.

Measured: 79233 ns/iteration steady-state (For_i repeat-loop differencing,
loop_reps 100 vs 1100), rel err 1.0e-3 vs the fp32 reference.
"""

import sys

sys.path.insert(0, "/opt/trn_rl_repo")

import numpy as np

import concourse.bass as bass
import concourse.mybir as mybir
import concourse.tile as tile
from concourse.bass_utils import run_bass_kernel_spmd

F32 = mybir.dt.float32
BF16 = mybir.dt.bfloat16
H = W = 256
NIMG = 2
NG = 64
NR = NG + 1
N_ACC = 4
S_TOT = 516       # 512 data slots + sentinels, uniform across partitions
SENT = -100.0
NCHUNK = 6
CW = S_TOT // NCHUNK  # 86 slot columns per elementwise chunk
DVC = 64          # sub columns on DVE per chunk (rest on gpsimd; GPSIMD measures ~2ns/el so DVE takes most)


def _build_program(split=True, mm_dtype=BF16, loop_reps=0, dvc=None):
    import contextlib

    dvc = DVC if dvc is None else dvc

    nc = bass.Bass()
    xs = nc.declare_dram_parameter("xsd", [NIMG, 128, S_TOT], F32, isOutput=False)
    shift = nc.declare_dram_parameter("shift", [128, NR], F32, isOutput=False)
    out = nc.declare_dram_parameter(
        "glcm", [NIMG, NR, N_ACC, NR, 2], F32, isOutput=True
    )

    # stationary slots: odd s' (1..515); acc chains split for fp32 accuracy
    sps = list(range(1, S_TOT, 2))
    acc_of = lambda sp: ((sp // 2) % 2) + 2 * (sp >= 258)
    totals = [0] * N_ACC
    for sp in sps:
        totals[acc_of(sp)] += 1

    with tile.TileContext(nc) as tc:
        with (
            tc.tile_pool(name="const", bufs=1) as const_pool,
            tc.tile_pool(name="xp", bufs=2) as x_pool,
            tc.tile_pool(name="arg", bufs=2) as arg_pool,
            tc.tile_pool(name="sig", bufs=2) as sig_pool,
            tc.tile_pool(name="oub", bufs=2) as out_pool,
            tc.tile_pool(name="ps", bufs=2, space="PSUM") as psum_pool,
        ):
            shift_raw = const_pool.tile([128, NR], F32)
            nc.sync.dma_start(shift_raw[:], shift[:])
            sh2 = const_pool.tile([128, NR], F32)
            nc.vector.tensor_copy(sh2[:], shift_raw[:])

            rep_ctx = (
                tc.For_i(0, loop_reps, 1) if loop_reps else contextlib.nullcontext()
            )
            with rep_ctx:
              for img in range(NIMG):
                psums = [
                    psum_pool.tile([NR, NR, 3], F32, tag=f"ps{g}", name=f"ps{g}")
                    for g in range(N_ACC)
                ]
                acc_mm = [0] * N_ACC

                xr = x_pool.tile([128, S_TOT], F32, tag="xr", name="xr")
                nc.sync.dma_start(xr[:], xs[img])
                xt = x_pool.tile([128, S_TOT], F32, tag="xt", name="xt")
                nc.vector.tensor_copy(xt[:], xr[:])
                A = sig_pool.tile([128, NR, S_TOT], mm_dtype, tag="A", name="A")
                for j in range(NCHUNK):
                    c0 = CW * j
                    arg = arg_pool.tile([128, NR, CW], F32, tag="arg", name="arg")
                    for (lo, hi, eng) in ((0, dvc, nc.vector), (dvc, CW, nc.gpsimd)):
                        ncol = hi - lo
                        xb = (
                            xt[:, c0 + lo : c0 + hi]
                            .unsqueeze(1)
                            .broadcast_to([128, NR, ncol])
                        )
                        shb = sh2[:, :].unsqueeze(2).broadcast_to([128, NR, ncol])
                        eng.tensor_sub(arg[:, :, lo:hi], xb, shb)
                    nc.scalar.activation(
                        A[:, :, c0 : c0 + CW],
                        arg[:, :, 0:CW],
                        mybir.ActivationFunctionType.Sigmoid,
                        scale=640.0,
                    )
                    # stationary s' usable once slots <= s'+1 are built:
                    # chunk j covers up to slot 86(j+1)-1
                    smax = CW * (j + 1) - 1
                    smin = CW * j - 1  # previous chunk handled s'+1 <= smin
                    for sp in sps:
                        need = sp + 1 if sp + 1 < S_TOT else sp
                        if not (smin < need <= smax):
                            continue
                        acc = acc_of(sp)
                        if sp + 1 < S_TOT:
                            nc.tensor.matmul(
                                psums[acc][:, :, :],
                                A[0:128, :, sp],
                                A[0:128, :, sp - 1 : sp + 2],
                                start=(acc_mm[acc] == 0),
                                stop=(acc_mm[acc] == totals[acc] - 1),
                            )
                        else:
                            # s'=515: only the ST half (pair 514)
                            nc.tensor.matmul(
                                psums[acc][:, :, 0],
                                A[0:128, :, sp],
                                A[0:128, :, sp - 1],
                                start=(acc_mm[acc] == 0),
                                stop=(acc_mm[acc] == totals[acc] - 1),
                            )
                        acc_mm[acc] += 1
                ob = out_pool.tile([NR, N_ACC, NR, 2], F32, name="ob")
                for g in range(N_ACC):
                    # keep halves h=0 (S^T) and h=2 (S); drop the self column
                    nc.vector.tensor_copy(ob[:, g, :, 0], psums[g][:, :, 0])
                    nc.vector.tensor_copy(ob[:, g, :, 1], psums[g][:, :, 2])
                nc.sync.dma_start(out[img], ob[:])
    if split:
        _split_waits(nc)
    return nc


def _split_waits(nc):
    n = 0
    for bb in nc.m.functions[0].blocks:
        out = []
        for ins in bb.instructions:
            si = ins.sync_info
            if si is not None and si.on_wait and len(si.on_wait) > 1:
                waits = list(si.on_wait)
                for w in waits[:-1]:
                    out.append(
                        mybir.InstDrain(
                            name=f"waitsplit-{n}",
                            engine=ins.engine,
                            sync_info=mybir.SyncInfo(on_wait=[w], on_update=[]),
                        )
                    )
                    n += 1
                ins.sync_info = mybir.SyncInfo(
                    on_wait=waits[-1:], on_update=list(si.on_update or [])
                )
            out.append(ins)
        bb.instructions[:] = out
    return n


def _shift_vec():
    sv = np.arange(0, NR, dtype=np.float64) / np.float64(NG)
    sv[0] = -10.0
    sv[NG] = 11.0
    return sv


def _diag_plan():
    """Static slot plan: for each partition p, the list of (flat_index or -1)
    of length S_TOT (-1 = sentinel). Flat index = r*W + c."""
    plan = np.full((128, S_TOT), -1, dtype=np.int64)
    for p in range(128):
        pos = 0
        for D in range(-255, 256):
            if D % 128 != p:
                continue
            cmin, cmax = max(0, -D), min(W - 1, W - 1 - D)
            pos += 1  # sentinel before each segment
            for c in range(cmin, cmax + 1):
                plan[p, pos] = (D + c) * W + c
                pos += 1
        assert pos <= S_TOT, pos
    return plan


_PLAN = _diag_plan()
_PLAN_MASK = _PLAN >= 0
_PLAN_IDX = np.where(_PLAN_MASK, _PLAN, 0)


def _to_diag(x):
    """x: [B, H, W] float32 -> [B, 128, S_TOT] with sentinels."""
    flat = x.reshape(x.shape[0], H * W)
    g = flat[:, _PLAN_IDX.reshape(-1)].reshape(x.shape[0], 128, S_TOT)
    g = np.where(_PLAN_MASK[None], g, np.float32(SENT))
    return np.ascontiguousarray(g.astype(np.float32))


def make_in_maps(x):
    sv = _shift_vec().astype(np.float32)
    shift = np.ascontiguousarray(np.broadcast_to(sv[None, :], (128, NR)))
    xd = _to_diag(x)
    return [
        {"xsd": np.ascontiguousarray(xd[2 * k : 2 * k + 2]), "shift": shift}
        for k in range(8)
    ]


def _finish_host(raw, x=None):
    # raw: [16, NR, N_ACC, NR, 2]; half 0 accumulated S^T, half 1 S
    r = raw.astype(np.float64).sum(axis=2)  # [16, NR, NR, 2]
    s = r[..., 0].transpose(0, 2, 1) + r[..., 1]
    g = s[:, :NG, :NG] - s[:, 1:, :NG] - s[:, :NG, 1:] + s[:, 1:, 1:]
    g = g / g.sum(axis=(1, 2), keepdims=True)
    return g.astype(np.float32)


def _postprocess(results, x):
    raw = np.concatenate([r["glcm"] for r in results], axis=0)
    return _finish_host(raw).reshape(16, 1, NG, NG, 1)


_NC = None


def kernel(x, offset_r=1, offset_c=1, **_):
    global _NC
    assert int(offset_r) == 1 and int(offset_c) == 1
    x = np.ascontiguousarray(np.asarray(x, dtype=np.float32).reshape(16, H, W))
    if _NC is None:
        _NC = _build_program()
    res = run_bass_kernel_spmd(_NC, make_in_maps(x), core_ids=list(range(8)))
    return _postprocess(res.results, x)


if __name__ == "__main__":
    _build_program()
    print("build OK")

